# revision 1
# baseline (speedup 1.0000x reference)
"""Trainium2 Bass kernel for MLP-projected multi-head attention + max-pool.

Problem (hardcoded shapes):
  x [4, 2048, 64] f32; q/k/v = MLP_m(x) with MLP(x) = elu(x@W1+b1)@W2+b2,
  W1 [64,256], W2 [256,128]; attention with H=8 heads, dk=16;
  out = max over seq of attention output -> [4, 128] f32.

Sharding: 8 cores = 4 batches x 2 head-groups (4 heads each). Each core
computes its batch's QKV MLP (full hidden layer, its 64 columns of the
second layer), attention for its 4 heads, and a [64]-wide slice of the
output row. No collectives; host gathers the 8 slices.

Key device-side design choices:
  * All big matmuls use fp16 operands (fp32 streams at 4 cyc/row on TRN2,
    fp16 at 1); PSUM accumulation stays fp32.
  * Everything is computed in transposed ("feature-major") layout so that
    softmax's exp is the single PSUM->SBUF pass over the big score matrix
    and the AV matmul can consume exp(S) directly from SBUF.
  * Biases are folded in as ones-row contractions; the attention 1/4 scale
    is folded into W2_q; ELU uses elu(z)+1 = max(z+1, min(exp(z),1)) with
    the +1 shift folded into the layer-2 effective bias.
  * Softmax denominator Z comes for free from a ones-column appended to v.
  * 4 heads run concurrently on the PE via tile_position row/col packing.
"""

import sys

import numpy as np

try:
    import concourse  # noqa: F401  (provided by the environment, e.g. axon site)
except ImportError:
    sys.path.insert(0, "/opt/trn_rl_repo")

B, S, F = 4, 2048, 64
HID, D, H = 256, 128, 8
DK = D // H          # 16
NHPC = 4             # heads per core
NCORES = 8
SQC = 512            # sq chunk width in phase C
NSQC = S // SQC      # 4
NKT = S // 128       # 16 sk tiles
F16 = np.float16

_nc_cache = {}


def _build_bass():
    import concourse.mybir as mybir
    import concourse.tile as tile
    from concourse import bacc

    f16, f32 = mybir.dt.float16, mybir.dt.float32
    Alu = mybir.AluOpType
    Act = mybir.ActivationFunctionType

    nc = bacc.Bacc()

    xta_d = nc.dram_tensor("xta", [F + 1, S], f16, kind="ExternalInput")
    w1a_d = nc.dram_tensor("w1a", [3, F + 1, HID], f16, kind="ExternalInput")
    # q/k second layer, arranged to 113 output partitions (head j at 32j..32j+16),
    # rows 0..255 = W2 (q pre-scaled by 1/4), row 256 = effective bias.
    wqk_d = nc.dram_tensor("wqk", [2, HID + 1, 113], f16, kind="ExternalInput")
    # v second layer, arranged to 68 cols (head j dims at 17j.., ones col at 17j+16)
    wv_d = nc.dram_tensor("wv", [HID + 1, 68], f16, kind="ExternalInput")
    sel_d = nc.dram_tensor("sel", [2, 2, 128], f32, kind="ExternalInput")
    selz_d = nc.dram_tensor("selz", [128, NHPC], f32, kind="ExternalInput")
    out_d = nc.dram_tensor("o", [NHPC * DK], f32, kind="ExternalOutput")

    with tile.TileContext(nc) as tc:
        with (
            tc.tile_pool(name="consts", bufs=1) as consts,
            tc.tile_pool(name="h1pool", bufs=6) as h1p,
            tc.tile_pool(name="qkt", bufs=2) as qktp,
            tc.tile_pool(name="v4pool", bufs=1) as v4p,
            tc.tile_pool(name="elu_e", bufs=2) as ep,
            tc.tile_pool(name="ptpool", bufs=3) as ptp,
            tc.tile_pool(name="episb", bufs=3) as epp,
            tc.tile_pool(name="res", bufs=1) as resp,
        ):
            # ---- load constants/weights ----
            xta = consts.tile([F + 1, S], f16)
            nc.sync.dma_start(out=xta, in_=xta_d[:, :])
            w1 = []
            for m in range(3):
                w1m = consts.tile([F + 1, HID], f16, name=f"w1_{m}")
                nc.sync.dma_start(out=w1m, in_=w1a_d[m, :, :])
                w1.append(w1m)
            wqk = []
            for m in range(2):
                a = consts.tile([128, 113], f16, name=f"wqkA_{m}")
                b = consts.tile([128, 113], f16, name=f"wqkB_{m}")
                cbias = consts.tile([1, 113], f16, name=f"wqkC_{m}")
                nc.sync.dma_start(out=a, in_=wqk_d[m, 0:128, :])
                nc.sync.dma_start(out=b, in_=wqk_d[m, 128:256, :])
                nc.sync.dma_start(out=cbias, in_=wqk_d[m, 256:257, :])
                wqk.append((a, b, cbias))
            wvA = consts.tile([128, 68], f16)
            wvB = consts.tile([128, 68], f16)
            wvC = consts.tile([1, 68], f16)
            nc.sync.dma_start(out=wvA, in_=wv_d[0:128, :])
            nc.sync.dma_start(out=wvB, in_=wv_d[128:256, :])
            nc.sync.dma_start(out=wvC, in_=wv_d[256:257, :])
            sel = consts.tile([2, 2, 128], f32)
            nc.sync.dma_start(out=sel, in_=sel_d[:, :, :])
            selz = consts.tile([128, NHPC], f32)
            nc.sync.dma_start(out=selz, in_=selz_d[:, :])
            ones = consts.tile([1, S], f16)
            nc.vector.memset(ones, 1.0)
            neg1 = consts.tile([128, 1], f32)
            nc.vector.memset(neg1, -1.0)

            # ---- phase A: layer 1 + ELU (h1' = elu(z)+1, fp16, transposed) ----
            h1 = [[None, None] for _ in range(3)]
            with tc.tile_pool(name="zb_ps", bufs=2, space="PSUM") as zbp:
                for m in range(3):
                    for ht in range(2):
                        zb = zbp.tile([128, S], f32)
                        for sc in range(4):
                            cs = slice(sc * 512, (sc + 1) * 512)
                            nc.tensor.matmul(
                                zb[:, cs],
                                lhsT=w1[m][:, ht * 128:(ht + 1) * 128],
                                rhs=xta[:, cs],
                                start=True, stop=True,
                            )
                        e = ep.tile([128, S], f16, tag="elu_e")
                        # zb holds z + b1 + 1; e = exp(z + b1)
                        nc.scalar.activation(e, zb, Act.Exp, bias=neg1[:, 0:1])
                        h1t = h1p.tile([128, S], f16, tag="h1", name=f"h1_{m}_{ht}")
                        # h1' = max(min(exp(z), 1), z + 1) = elu(z) + 1
                        nc.vector.scalar_tensor_tensor(
                            out=h1t, in0=e, scalar=1.0, in1=zb,
                            op0=Alu.min, op1=Alu.max,
                        )
                        h1[m][ht] = h1t

            # ---- phase B: layer 2 -> qT/kT (113 parts x S) and v (S x 68) ----
            qkT = []
            with (
                tc.tile_pool(name="qk_ps", bufs=1, space="PSUM") as qkps,
                tc.tile_pool(name="v_ps", bufs=2, space="PSUM") as vps,
            ):
                for m in range(2):
                    ps = qkps.tile([113, S], f32, tag="qkps")
                    for sc in range(4):
                        cs = slice(sc * 512, (sc + 1) * 512)
                        nc.tensor.matmul(ps[:, cs], lhsT=wqk[m][0],
                                         rhs=h1[m][0][:, cs], start=True, stop=False)
                        nc.tensor.matmul(ps[:, cs], lhsT=wqk[m][1],
                                         rhs=h1[m][1][:, cs], start=False, stop=False)
                        nc.tensor.matmul(ps[:, cs], lhsT=wqk[m][2],
                                         rhs=ones[:, cs], start=False, stop=True)
                    qt = qktp.tile([113, S], f16, tag="qkt", name=f"qkT_{m}")
                    nc.vector.tensor_copy(qt, ps)
                    qkT.append(qt)
                v4 = v4p.tile([128, NKT * 68], f16)
                for st in range(NKT):
                    ss = slice(st * 128, (st + 1) * 128)
                    vp = vps.tile([128, 68], f32, tag="vps")
                    nc.tensor.matmul(vp, lhsT=h1[2][0][:, ss], rhs=wvA,
                                     start=True, stop=False)
                    nc.tensor.matmul(vp, lhsT=h1[2][1][:, ss], rhs=wvB,
                                     start=False, stop=False)
                    nc.tensor.matmul(vp, lhsT=ones[:, ss], rhs=wvC,
                                     start=False, stop=True)
                    nc.vector.tensor_copy(v4[:, st * 68:(st + 1) * 68], vp)

            # ---- phase C: attention, 2-head pairs, sq chunks of SQC ----
            # Concurrent row-packed score MMs must drain into different PSUM
            # banks, so each head's [128, 512] score block gets its own bank.
            omaxp = [resp.tile([128, 1], f32, name=f"omax{p}") for p in range(2)]
            with (
                tc.tile_pool(name="s2_ps", bufs=2, space="PSUM") as s2p,
                tc.tile_pool(name="acc_ps", bufs=4, space="PSUM") as accp,
            ):
                for hp in range(2):
                    heads = [2 * hp, 2 * hp + 1]
                    for c in range(NSQC):
                        sq = slice(c * SQC, (c + 1) * SQC)
                        nt = [accp.tile([128, SQC], f32, tag="nt",
                                        name=f"nt{hp}_{c}_{j}") for j in range(2)]
                        for t in range(NKT):
                            ts_ = slice(t * 128, (t + 1) * 128)
                            s2 = s2p.tile([128, 2 * SQC], f32, tag="s2")
                            for j, h in enumerate(heads):
                                hs = slice(32 * h, 32 * h + DK)
                                nc.tensor.matmul(
                                    s2[:, j * SQC:(j + 1) * SQC],
                                    lhsT=qkT[1][hs, ts_], rhs=qkT[0][hs, sq],
                                    start=True, stop=True,
                                    tile_position=(32 * h, 0),
                                )
                            pt = ptp.tile([128, 2 * SQC], f16, tag="pt")
                            nc.scalar.activation(pt, s2, Act.Exp)
                            for j, h in enumerate(heads):
                                nc.tensor.matmul(
                                    nt[j][32 * h:32 * h + DK + 1, :],
                                    lhsT=v4[:, t * 68 + 17 * h:t * 68 + 17 * h + 17],
                                    rhs=pt[:, j * SQC:(j + 1) * SQC],
                                    start=(t == 0), stop=(t == NKT - 1),
                                    tile_position=(0, 32 * h),
                                )
                        # epilogue: out_h = NT_h / Z_h, running max over sq
                        ntsb = epp.tile([128, SQC], f32, tag="ntsb")
                        nc.vector.memset(ntsb, 0.0)
                        for j, h in enumerate(heads):
                            hp17 = slice(32 * h, 32 * h + DK + 1)
                            nc.vector.tensor_copy(ntsb[hp17, :], nt[j][hp17, :])
                        # gather the pair's Z rows (partitions 32h+16) to rows 0..1
                        zc = accp.tile([2, SQC], f32, tag="nt", name=f"zc{hp}_{c}")
                        nc.tensor.matmul(zc, lhsT=selz[:, 2 * hp:2 * hp + 2],
                                         rhs=ntsb, start=True, stop=True)
                        rz = epp.tile([2, SQC], f32, tag="rz")
                        nc.vector.reciprocal(rz, zc)
                        rzb = accp.tile([128, SQC], f32, tag="nt",
                                        name=f"rzb{hp}_{c}")
                        nc.tensor.matmul(rzb, lhsT=sel[:, hp, :],
                                         rhs=rz, start=True, stop=True)
                        prod = epp.tile([128, SQC], f32, tag="prod")
                        cmax = epp.tile([128, 1], f32, tag="cmax")
                        nc.vector.tensor_mul(prod, ntsb, rzb)
                        nc.vector.tensor_reduce(
                            cmax, prod, axis=mybir.AxisListType.X, op=Alu.max)
                        if c == 0:
                            nc.vector.tensor_copy(omaxp[hp], cmax)
                        else:
                            nc.vector.tensor_max(omaxp[hp], omaxp[hp], cmax)
            for h in range(NHPC):
                nc.sync.dma_start(
                    out=out_d[h * DK:(h + 1) * DK],
                    in_=omaxp[h // 2][32 * h:32 * h + DK, 0:1],
                )
    nc.compile()
    return nc


def _prep_inputs(inputs):
    """Host-side sharding + layout staging (weights/activations -> fp16)."""
    x = np.asarray(inputs["x"], np.float32)
    W1 = [np.asarray(inputs[m + "W1"], np.float32) for m in "qkv"]
    b1 = [np.asarray(inputs[m + "b1"], np.float32) for m in "qkv"]
    W2 = [np.asarray(inputs[m + "W2"], np.float32) for m in "qkv"]
    b2 = [np.asarray(inputs[m + "b2"], np.float32) for m in "qkv"]

    w1a = np.zeros((3, F + 1, HID), F16)
    for m in range(3):
        w1a[m, :F] = W1[m].astype(F16)
        w1a[m, F] = (b1[m] + 1.0).astype(F16)

    # effective bias absorbs the h1' = elu+1 shift: b2eff = b2 - colsum(fp16(W2))
    W2h = [w.astype(F16) for w in W2]
    b2eff = [b2[m] - W2h[m].astype(np.float32).sum(axis=0) for m in range(3)]

    sel = np.zeros((2, 2, 128), np.float32)
    selz = np.zeros((128, NHPC), np.float32)
    for j in range(NHPC):
        sel[j % 2, j // 2, 32 * j:32 * (j + 1)] = 1.0
        selz[32 * j + DK, j] = 1.0

    in_maps = []
    for c in range(NCORES):
        b, hg = c // 2, c % 2
        heads = [NHPC * hg + j for j in range(NHPC)]

        xta = np.zeros((F + 1, S), F16)
        xta[:F] = x[b].T.astype(F16)
        xta[F] = 1.0

        wqk = np.zeros((2, HID + 1, 113), F16)
        for i in range(2):  # 0=q (scaled), 1=k
            scale = 0.25 if i == 0 else 1.0
            for j, hh in enumerate(heads):
                cols = slice(hh * DK, (hh + 1) * DK)
                wqk[i, :HID, 32 * j:32 * j + DK] = (
                    W2h[i][:, cols].astype(np.float32) * scale).astype(F16)
                wqk[i, HID, 32 * j:32 * j + DK] = (
                    b2eff[i][cols] * scale).astype(F16)

        wv = np.zeros((HID + 1, 68), F16)
        for j, hh in enumerate(heads):
            cols = slice(hh * DK, (hh + 1) * DK)
            wv[:HID, 17 * j:17 * j + DK] = W2h[2][:, cols]
            wv[HID, 17 * j:17 * j + DK] = b2eff[2][cols].astype(F16)
            wv[HID, 17 * j + DK] = 1.0  # ones column -> softmax denominator

        in_maps.append({"xta": xta, "w1a": w1a, "wqk": wqk, "wv": wv,
                        "sel": sel, "selz": selz})
    return in_maps


def kernel(**inputs):
    from concourse import bass_utils

    if "nc" not in _nc_cache:
        _nc_cache["nc"] = _build_bass()
    nc = _nc_cache["nc"]
    in_maps = _prep_inputs(inputs)
    res = bass_utils.run_bass_kernel_spmd(nc, in_maps, core_ids=list(range(NCORES)))
    out = np.zeros((B, D), np.float32)
    for c in range(NCORES):
        b, hg = c // 2, c % 2
        out[b, hg * 64:(hg + 1) * 64] = res.results[c]["o"]
    return out


if __name__ == "__main__":
    rng = np.random.default_rng(0)
    ins = {"x": rng.standard_normal((B, S, F), dtype=np.float32)}
    for m in "qkv":
        s1, s2 = 1 / np.sqrt(F), 1 / np.sqrt(HID)
        ins[m + "W1"] = rng.uniform(-s1, s1, (F, HID)).astype(np.float32)
        ins[m + "b1"] = rng.uniform(-s1, s1, (HID,)).astype(np.float32)
        ins[m + "W2"] = rng.uniform(-s2, s2, (HID, D)).astype(np.float32)
        ins[m + "b2"] = rng.uniform(-s2, s2, (D,)).astype(np.float32)
    print(kernel(**ins)[:, :4])

